# revision 24
# baseline (speedup 1.0000x reference)
"""Trainium2 Bass kernel for nn_MultiScaleDHSM (multi-scale diagonal-SSM LM block).

Strategy (zero-communication SPMD over 8 cores):
  core c owns tokens [512*q, 512*(q+1)) of batch b, where b=c//4, q=c%4.
  Device keeps everything feature-major [feature, token] so every per-feature
  vector (biases, D, LN folds) is a per-partition scalar.  The sequential
  recurrence s_t = A*s_{t-1} + u_t runs on the HW tensor_tensor_scan op over
  the core's right-aligned token prefix (front zero-padded so the 512-token
  "own window" sits at a static offset in an identical program on all cores).
  LayerNorm stats (reduction over features = partitions) are computed with a
  ones-matmul on the PE, which both reduces and broadcasts across partitions.
  LN scales fold into the following matmul weights on the host (g -> Wo,
  gf -> Wh); per-feature biases are applied as per-partition ACT biases.
"""

import os
from contextlib import ExitStack

import ml_dtypes
import numpy as np

import concourse.bass as bass
import concourse.mybir as mybir
import concourse.tile as tile
from concourse import bacc
from concourse.bass import ds, ts
from concourse.bass_utils import run_bass_kernel_spmd

B, S, H, V = 2, 2048, 1024, 32000
SDS = [64, 128, 256]
NP = 512  # packed state dim: [L0:0-64 | pad:64-128 | L1:128-256 | L2:256-512]
SLOT = [0, 128, 256]
NT_OF_LAYER = [[0], [1], [2, 3]]  # which 128-row n-tiles belong to each layer
TPC = 512  # tokens per core
EPS = 1e-5
VG = 1000  # head vocab group (streamed Wh slice width)
VC = 500   # head vocab chunk (one PSUM bank, <=512 fp32)
NGRP = V // VG
NCHK = VG // VC

F32 = mybir.dt.float32
BF16 = mybir.dt.bfloat16
BF = ml_dtypes.bfloat16
AF = mybir.ActivationFunctionType
OP = mybir.AluOpType

last_exec_time_ns = None
last_bass_results = None

_prog_cache = {}


def _layernorm_stats(nc, psum, sb, ones, epst, ys, sqs, suf):
    """ys/sqs: lists of 8 [128, TPC] bf16 tiles. Returns (rstd, mur) [128, TPC] f32."""
    pm1 = psum.tile([128, TPC], F32, tag="ps", bufs=8, name=f"pm1{suf}")
    pm2 = psum.tile([128, TPC], F32, tag="ps", bufs=8, name=f"pm2{suf}")
    for ho in range(8):
        nc.tensor.matmul(pm1[:], ones[:], ys[ho][:], start=(ho == 0), stop=(ho == 7))
    for ho in range(8):
        nc.tensor.matmul(pm2[:], ones[:], sqs[ho][:], start=(ho == 0), stop=(ho == 7))
    musq = sb.tile([128, TPC], F32, tag="musq", bufs=2, name=f"musq{suf}")
    nc.scalar.activation(musq[:], pm1[:], AF.Square, scale=1.0 / H)
    var = sb.tile([128, TPC], F32, tag="var", bufs=2, name=f"var{suf}")
    nc.vector.scalar_tensor_tensor(var[:], pm2[:], 1.0 / H, musq[:], OP.mult, OP.subtract)
    sd = sb.tile([128, TPC], F32, tag="sd", bufs=2, name=f"sd{suf}")
    nc.scalar.activation(sd[:], var[:], AF.Sqrt, bias=epst[:, 0:1])
    rstd = sb.tile([128, TPC], F32, tag="rstd", bufs=2, name=f"rstd{suf}")
    nc.vector.reciprocal(rstd[:], sd[:])
    mur = sb.tile([128, TPC], F32, tag="mur", bufs=2, name=f"mur{suf}")
    nc.vector.scalar_tensor_tensor(mur[:], pm1[:], 1.0 / H, rstd[:], OP.mult, OP.mult)
    return rstd, mur


def _body(tc, io):
    nc = tc.nc
    with ExitStack() as ctx:
        sb = ctx.enter_context(tc.tile_pool(name="sb", bufs=1))
        sb2 = ctx.enter_context(tc.tile_pool(name="sb2", bufs=2))
        sb3 = ctx.enter_context(tc.tile_pool(name="sb3", bufs=3))
        psum = ctx.enter_context(tc.tile_pool(name="ps", bufs=4, space="PSUM"))

        r8 = lambda ap: ap.rearrange("(r p) t -> p r t", p=128)

        def dma_in(pool, name, src_ap, shape, dtype, bufs=None):
            kw = {"bufs": bufs} if bufs else {}
            t = pool.tile(shape, dtype, tag=name, name=name, **kw)
            nc.sync.dma_start(t[:], src_ap)
            return t

        # ---- persistent small tensors (stage-1 needs first; rest after) ----
        wgb = sb2.tile([128, 8, 2 * NP], BF16, tag="w16", bufs=3, name="wgb")
        for r in range(8):
            nc.sync.dma_start(wgb[:, r, :], r8(io["wgb"])[:, r, :])
        bgp = dma_in(sb, "bgp", io["bgp"].rearrange("(n p) o -> p (n o)", p=128), [128, 4], F32)
        apk = dma_in(sb, "apk", io["apk"].rearrange("(n p) o -> p (n o)", p=128), [128, 4], F32)
        ones = sb.tile([128, 128], BF16, tag="ones")
        nc.gpsimd.memset(ones[:], 1.0)
        epst = sb.tile([128, 1], F32, tag="epst")
        nc.gpsimd.memset(epst[:], EPS)

        # ---- stage 1+2: u = sigmoid(emb@WgT + bg) * (emb@WbT); chained HW scan ----
        states = None
        prev_states = None
        embm = None  # the t4=3 chunk doubles as the own-window embedding
        for t4 in range(4):
            if t4 < 3:
                et = sb2.tile([128, 8, 512], BF16, tag="e8", name=f"et{t4}")
            else:
                et = sb.tile([128, 8, 512], BF16, tag="embm", name="et3")
                embm = et
            for r in range(8):
                nc.sync.dma_start(et[:, r, :], r8(io["embt"])[:, r, ts(t4, 512)])
            st = sb2.tile([128, 4, 512], BF16, tag="stc", name=f"st{t4}")
            for nt in range(4):
                pg = psum.tile([128, 512], F32, tag="ps", bufs=8, name=f"pg{t4}_{nt}")
                pb = psum.tile([128, 512], F32, tag="ps", bufs=8, name=f"pb{t4}_{nt}")
                for r in range(8):
                    nc.tensor.matmul(pg[:], wgb[:, r, ts(nt, 128)], et[:, r, :],
                                     start=(r == 0), stop=(r == 7))
                for r in range(8):
                    nc.tensor.matmul(pb[:], wgb[:, r, ts(4 + nt, 128)], et[:, r, :],
                                     start=(r == 0), stop=(r == 7))
                gate = sb2.tile([128, 512], BF16, tag="gate")
                nc.scalar.activation(gate[:], pg[:], AF.Sigmoid, bias=bgp[:, nt:nt + 1])
                uc = sb3.tile([128, 512], BF16, tag="uc")
                nc.vector.tensor_mul(uc[:], gate[:], pb[:])
                init = 0.0 if t4 == 0 else prev_states[:, nt, 511:512]
                nc.vector.tensor_tensor_scan(st[:, nt, :],
                                             apk[:, nt:nt + 1].to_broadcast([128, 512]),
                                             uc[:], init, OP.mult, OP.add)
            prev_states = st
        states = prev_states  # [128, 4, 512] bf16: my-window states

        # remaining persistent small tensors (first used in stage 3/4)
        wct = dma_in(sb, "wct", io["wct"].rearrange("(n p) h -> p n h", p=128), [128, 4, H], BF16)
        d1 = dma_in(sb, "d1", r8(io["d1"]), [128, 8, 3], F32)
        bop = dma_in(sb, "bop", r8(io["bop"]), [128, 8, 3], F32)
        bfv = dma_in(sb, "bfv", r8(io["bfv"]), [128, 8, 1], F32)

        # ---- stage 3: per layer: y = states@WcT + (D+1)*x ; LN1 ; o = normed@Wo' ----
        cmb = []  # 24 per-k [128, TPC] bf16 tiles (fine-grained deps into Wf)
        for i in range(3):
            ys, sqs = [], []
            tls = NT_OF_LAYER[i]
            for ho in range(8):
                py = psum.tile([128, TPC], F32, tag="ps", bufs=8, name=f"py{i}_{ho}")
                for j, nt in enumerate(tls):
                    nc.tensor.matmul(py[:], wct[:, nt, ts(ho, 128)], states[:, nt, :],
                                     start=(j == 0), stop=(j == len(tls) - 1))
                y = sb2.tile([128, TPC], BF16, tag="yt", bufs=12, name=f"y{i}_{ho}")
                nc.vector.scalar_tensor_tensor(y[:], embm[:, ho, :], d1[:, ho, i:i + 1],
                                               py[:], OP.mult, OP.add)
                s = sb2.tile([128, TPC], BF16, tag="sqt", bufs=11, name=f"s{i}_{ho}")
                nc.scalar.activation(s[:], y[:], AF.Square)
                ys.append(y)
                sqs.append(s)
            rstd, mur = _layernorm_stats(nc, psum, sb, ones, epst, ys, sqs, f"l{i}")
            nrm = []
            for ho in range(8):
                tmp = sb2.tile([128, TPC], F32, tag="tmp", bufs=3, name=f"tp{i}_{ho}")
                nc.vector.tensor_mul(tmp[:], ys[ho][:], rstd[:])
                nv = sb2.tile([128, TPC], BF16, tag="nrmt", bufs=12, name=f"n{i}_{ho}")
                nc.vector.tensor_sub(nv[:], tmp[:], mur[:])
                nrm.append(nv)
            for ho in range(8):
                woth = sb2.tile([128, 24, 128], BF16, tag="wst", name=f"woth{i}_{ho}")[:, :8, :]
                nc.sync.dma_start(woth[:], r8(io["wot"])[:, :, ds(i * H + ho * 128, 128)])
                po = psum.tile([128, TPC], F32, tag="ps", bufs=8, name=f"po{i}_{ho}")
                for hi in range(8):
                    nc.tensor.matmul(po[:], woth[:, hi, :], nrm[hi][:],
                                     start=(hi == 0), stop=(hi == 7))
                c = sb.tile([128, TPC], BF16, tag="cmb", bufs=24, name=f"cmb{i}_{ho}")
                nc.scalar.activation(c[:], po[:], AF.Identity, bias=bop[:, ho, i:i + 1])
                cmb.append(c)

        # ---- stage 4: fused = LN2(combined@WfT + bf + emb) ----
        fs, sq2s = [], []
        for ho in range(8):
            wfh = sb2.tile([128, 24, 128], BF16, tag="wst", name=f"wfh{ho}")
            nc.sync.dma_start(wfh[:], io["wft"].rearrange("(k p) m -> p k m", p=128)[:, :, ts(ho, 128)])
            pf = psum.tile([128, TPC], F32, tag="ps", bufs=8, name=f"pf{ho}")
            for k in range(24):
                nc.tensor.matmul(pf[:], wfh[:, k, :], cmb[k][:],
                                 start=(k == 0), stop=(k == 23))
            f = sb2.tile([128, TPC], BF16, tag="yt", bufs=12, name=f"f{ho}")
            nc.vector.scalar_tensor_tensor(f[:], pf[:], bfv[:, ho, 0:1],
                                           embm[:, ho, :], OP.add, OP.add)
            s2 = sb2.tile([128, TPC], BF16, tag="sqt", bufs=11, name=f"s4_{ho}")
            nc.scalar.activation(s2[:], f[:], AF.Square)
            fs.append(f)
            sq2s.append(s2)
        rstd2, mur2 = _layernorm_stats(nc, psum, sb, ones, epst, fs, sq2s, "f")
        norm2 = []
        for ho in range(8):
            tmp = sb2.tile([128, TPC], F32, tag="tmp", bufs=3, name=f"tp4_{ho}")
            nc.vector.tensor_mul(tmp[:], fs[ho][:], rstd2[:])
            nv = sb.tile([128, TPC], BF16, tag="n2", bufs=8, name=f"n2_{ho}")
            nc.vector.tensor_sub(nv[:], tmp[:], mur2[:])
            norm2.append(nv)

        # ---- stage 5: head: logits[t, v] = norm2.T @ Wh' (+ bh') ----
        bh_nonzero = io.get("bhp") is not None
        if bh_nonzero:
            e0row = sb.tile([128, 128], BF16, tag="e0row")
            nc.gpsimd.memset(e0row[:], 0.0)
            nc.gpsimd.memset(e0row[0:1, :], 1.0)
        for grp in range(NGRP):
            whg = sb2.tile([128, 8, VG], BF16, tag="w16", bufs=3, name=f"whg{grp}")
            nc.sync.dma_start(whg[:], r8(io["wht"])[:, :, ts(grp, VG)])
            if bh_nonzero:
                bhg = sb2.tile([128, VG], BF16, tag="bhg")
                nc.gpsimd.memset(bhg[:], 0.0)
                nc.sync.dma_start(bhg[0:1, :], io["bhp"][:, ts(grp, VG)])
            for t in range(4):
                osb = sb2.tile([128, VG], F32, tag="osb")
                pcs = [psum.tile([128, VC], F32, tag="ps", bufs=8, name=f"hc{grp}_{t}_{c}")
                       for c in range(NCHK)]
                for k in range(8):
                    for c in range(NCHK):
                        nc.tensor.matmul(pcs[c][:], norm2[k][:, ts(t, 128)], whg[:, k, ts(c, VC)],
                                         start=(k == 0), stop=(k == 7 and not bh_nonzero))
                if bh_nonzero:
                    for c in range(NCHK):
                        nc.tensor.matmul(pcs[c][:], e0row[:], bhg[:, ts(c, VC)],
                                         start=False, stop=True)
                for c in range(NCHK):
                    if c % 2 == 0:
                        nc.scalar.copy(osb[:, ts(c, VC)], pcs[c][:])
                    else:
                        nc.vector.tensor_copy(osb[:, ts(c, VC)], pcs[c][:])
                nc.sync.dma_start(io["out"][ts(t, 128), ts(grp, VG)], osb[:])


def _get_program(bh_nonzero):
    key = bool(bh_nonzero)
    if key in _prog_cache:
        return _prog_cache[key]
    nc = bacc.Bacc("TRN2", target_bir_lowering=False, debug=False, num_devices=8)
    io = {}

    def din(name, shape, dtype):
        io[name] = nc.dram_tensor(name, list(shape), dtype, kind="ExternalInput").ap()

    din("embt", (H, S), BF16)
    din("embm", (H, TPC), BF16)
    din("wgb", (H, 2 * NP), BF16)
    din("bgp", (NP, 1), F32)
    din("apk", (NP, 1), F32)
    din("wct", (NP, H), BF16)
    din("d1", (H, 3), F32)
    din("wot", (H, 3 * H), BF16)
    din("bop", (H, 3), F32)
    din("wft", (3 * H, H), BF16)
    din("bfv", (H, 1), F32)
    din("wht", (H, V), BF16)
    if bh_nonzero:
        din("bhp", (1, V), BF16)
    else:
        io["bhp"] = None
    io["out"] = nc.dram_tensor("out", [TPC, V], F32, kind="ExternalOutput").ap()

    with tile.TileContext(nc) as tc:
        _body(tc, io)
    nc.compile()
    _prog_cache[key] = nc
    return nc


def _prepare(inputs):
    f32 = np.float32
    E = np.asarray(inputs["E"], f32)
    x = np.asarray(inputs["x"]).astype(np.int64)
    emb = E[x]  # [B, S, H]

    Wg = [np.asarray(inputs[f"Wg{i}"], f32) for i in range(3)]
    Wb = [np.asarray(inputs[f"Wb{i}"], f32) for i in range(3)]
    Wc = [np.asarray(inputs[f"Wc{i}"], f32) for i in range(3)]
    Dv = [np.asarray(inputs[f"D{i}"], f32) for i in range(3)]
    bg = [np.asarray(inputs[f"bg{i}"], f32) for i in range(3)]
    Wo = [np.asarray(inputs[f"Wo{i}"], f32) for i in range(3)]
    bo = [np.asarray(inputs[f"bo{i}"], f32) for i in range(3)]
    gv = [np.asarray(inputs[f"g{i}"], f32) for i in range(3)]
    be = [np.asarray(inputs[f"be{i}"], f32) for i in range(3)]
    Alog = [np.asarray(inputs[f"A{i}"], f32) for i in range(3)]
    Wf = np.asarray(inputs["Wf"], f32)
    bf = np.asarray(inputs["bf"], f32)
    gf = np.asarray(inputs["gf"], f32)
    bef = np.asarray(inputs["bef"], f32)
    Wh = np.asarray(inputs["Wh"], f32)
    bh = np.asarray(inputs["bh"], f32)

    wgb = np.zeros((H, 2 * NP), f32)
    bgp = np.zeros((NP, 1), f32)
    apk = np.zeros((NP, 1), f32)
    wct = np.zeros((NP, H), f32)
    for i, N in enumerate(SDS):
        s = SLOT[i]
        wgb[:, s:s + N] = Wg[i].T
        wgb[:, NP + s:NP + s + N] = Wb[i].T
        bgp[s:s + N, 0] = bg[i]
        apk[s:s + N, 0] = np.exp(Alog[i])
        wct[s:s + N, :] = Wc[i].T
    d1 = np.stack([Dv[i] + 1.0 for i in range(3)], axis=1)  # [H, 3]
    wot = np.concatenate([(Wo[i] * gv[i][None, :]).T for i in range(3)], axis=1)  # [H, 3H]
    bop = np.stack([Wo[i] @ be[i] + bo[i] for i in range(3)], axis=1)  # [H, 3]
    wft = Wf.T.copy()  # [3H, H]
    wht = (Wh * gf[None, :]).T.copy()  # [H, V]
    bhp = Wh @ bef + bh  # [V]
    bh_nonzero = bool(np.abs(bhp).max() > 0)

    shared = {
        "wgb": wgb.astype(BF),
        "bgp": bgp,
        "apk": apk,
        "wct": wct.astype(BF),
        "d1": d1,
        "wot": wot.astype(BF),
        "bop": bop,
        "wft": wft.astype(BF),
        "bfv": bf.reshape(H, 1),
        "wht": wht.astype(BF),
    }
    if bh_nonzero:
        shared["bhp"] = bhp.reshape(1, V).astype(BF)

    in_maps = []
    for c in range(8):
        b, q = c // 4, c % 4
        plen = TPC * (q + 1)
        et = np.zeros((H, S), BF)
        et[:, S - plen:] = emb[b, :plen].T.astype(BF)
        em = np.ascontiguousarray(emb[b, q * TPC:(q + 1) * TPC].T).astype(BF)
        m = dict(shared)
        m["embt"] = et
        m["embm"] = em
        in_maps.append(m)
    return in_maps, bh_nonzero


def kernel(**inputs):
    global last_exec_time_ns, last_bass_results
    in_maps, bh_nonzero = _prepare(inputs)
    nc = _get_program(bh_nonzero)
    trace = os.environ.get("BASS_KERNEL_TRACE") == "1"
    tmpdir = os.environ.get("BASS_TRACE_DIR") or None
    res = run_bass_kernel_spmd(nc, in_maps, core_ids=list(range(8)), trace=trace,
                               tmpdir=tmpdir)
    last_exec_time_ns = res.exec_time_ns
    last_bass_results = res

    out = np.empty((B, S, V), np.float32)
    for c in range(8):
        b, q = c // 4, c % 4
        out[b, q * TPC:(q + 1) * TPC, :] = res.results[c]["out"]
    return out


# revision 34
# speedup vs baseline: 1.0501x; 1.0501x over previous
"""Trainium2 Bass kernel for nn_MultiScaleDHSM (multi-scale diagonal-SSM LM block).

Strategy (zero-communication SPMD over 8 cores):
  core c owns tokens [512*q, 512*(q+1)) of batch b, where b=c//4, q=c%4.
  Device keeps everything feature-major [feature, token] so every per-feature
  vector (biases, D, LN folds) is a per-partition scalar.  The sequential
  recurrence s_t = A*s_{t-1} + u_t runs on the HW tensor_tensor_scan op over
  the core's right-aligned token prefix (front zero-padded so the 512-token
  "own window" sits at a static offset in an identical program on all cores).
  LayerNorm stats (reduction over features = partitions) are computed with a
  ones-matmul on the PE, which both reduces and broadcasts across partitions.
  LN scales fold into the following matmul weights on the host (g -> Wo,
  gf -> Wh); per-feature biases are applied as per-partition ACT biases.
"""

import os
from contextlib import ExitStack

import ml_dtypes
import numpy as np

import concourse.bass as bass
import concourse.mybir as mybir
import concourse.tile as tile
from concourse import bacc
from concourse.bass import ds, ts
from concourse.bass_utils import run_bass_kernel_spmd
from concourse.masks import make_identity

B, S, H, V = 2, 2048, 1024, 32000
SDS = [64, 128, 256]
NP = 512  # packed state dim: [L0:0-64 | pad:64-128 | L1:128-256 | L2:256-512]
SLOT = [0, 128, 256]
NT_OF_LAYER = [[0], [1], [2, 3]]  # which 128-row n-tiles belong to each layer
TPC = 512  # tokens per core
EPS = 1e-5
VG = 1000  # head vocab group (streamed Wh slice width)
VC = 500   # head vocab chunk (one PSUM bank, <=512 fp32)
NGRP = V // VG
NCHK = VG // VC

F32 = mybir.dt.float32
BF16 = mybir.dt.bfloat16
BF = ml_dtypes.bfloat16
AF = mybir.ActivationFunctionType
OP = mybir.AluOpType

last_exec_time_ns = None
last_bass_results = None

_prog_cache = {}


def _layernorm_stats(nc, psum, sb, ones, epst, ys, sqs, suf):
    """ys/sqs: lists of 8 [128, TPC] bf16 tiles. Returns (rstd, mur) [128, TPC] f32."""
    pm1 = psum.tile([128, TPC], F32, tag="ps", bufs=8, name=f"pm1{suf}")
    pm2 = psum.tile([128, TPC], F32, tag="ps", bufs=8, name=f"pm2{suf}")
    for ho in range(8):
        nc.tensor.matmul(pm1[:], ones[:], ys[ho][:], start=(ho == 0), stop=(ho == 7))
    for ho in range(8):
        nc.tensor.matmul(pm2[:], ones[:], sqs[ho][:], start=(ho == 0), stop=(ho == 7))
    musq = sb.tile([128, TPC], F32, tag="musq", bufs=2, name=f"musq{suf}")
    nc.scalar.activation(musq[:], pm1[:], AF.Square, scale=1.0 / H)
    var = sb.tile([128, TPC], F32, tag="var", bufs=2, name=f"var{suf}")
    nc.vector.scalar_tensor_tensor(var[:], pm2[:], 1.0 / H, musq[:], OP.mult, OP.subtract)
    sd = sb.tile([128, TPC], F32, tag="sd", bufs=2, name=f"sd{suf}")
    nc.scalar.activation(sd[:], var[:], AF.Sqrt, bias=epst[:, 0:1])
    rstd = sb.tile([128, TPC], F32, tag="rstd", bufs=2, name=f"rstd{suf}")
    nc.vector.reciprocal(rstd[:], sd[:])
    mur = sb.tile([128, TPC], F32, tag="mur", bufs=2, name=f"mur{suf}")
    nc.vector.scalar_tensor_tensor(mur[:], pm1[:], 1.0 / H, rstd[:], OP.mult, OP.mult)
    return rstd, mur


def _body(tc, io):
    nc = tc.nc
    with ExitStack() as ctx:
        sb = ctx.enter_context(tc.tile_pool(name="sb", bufs=1))
        sb2 = ctx.enter_context(tc.tile_pool(name="sb2", bufs=2))
        sb3 = ctx.enter_context(tc.tile_pool(name="sb3", bufs=3))
        psum = ctx.enter_context(tc.tile_pool(name="ps", bufs=4, space="PSUM"))

        r8 = lambda ap: ap.rearrange("(r p) t -> p r t", p=128)

        def dma_in(pool, name, src_ap, shape, dtype, bufs=None):
            kw = {"bufs": bufs} if bufs else {}
            t = pool.tile(shape, dtype, tag=name, name=name, **kw)
            nc.sync.dma_start(t[:], src_ap)
            return t

        # ---- persistent small tensors (stage-1 needs first; rest after) ----
        wgb = sb2.tile([128, 8, 2 * NP], BF16, tag="w16", bufs=3, name="wgb")
        for r in range(8):
            nc.sync.dma_start(wgb[:, r, :], r8(io["wgb"])[:, r, :])
        bgp = dma_in(sb, "bgp", io["bgp"].rearrange("(n p) o -> p (n o)", p=128), [128, 4], F32)
        apk = dma_in(sb, "apk", io["apk"].rearrange("(n p) o -> p (n o)", p=128), [128, 4], F32)
        ones = sb.tile([128, 128], BF16, tag="ones")
        nc.gpsimd.memset(ones[:], 1.0)
        epst = sb.tile([128, 1], F32, tag="epst")
        nc.gpsimd.memset(epst[:], EPS)

        # ---- stage 1+2: u = sigmoid(emb@WgT + bg) * (emb@WbT); chained HW scan ----
        states = None
        prev_states = None
        embm = None  # the t4=3 chunk doubles as the own-window embedding
        for t4 in range(4):
            if t4 < 3:
                et = sb2.tile([128, 8, 512], BF16, tag="e8", name=f"et{t4}")
            else:
                et = sb.tile([128, 8, 512], BF16, tag="embm", name="et3")
                embm = et
            for r in range(8):
                nc.sync.dma_start(et[:, r, :], r8(io["embt"])[:, r, ts(t4, 512)])
            st = sb2.tile([128, 4, 512], BF16, tag="stc", name=f"st{t4}")
            for nt in range(4):
                pg = psum.tile([128, 512], F32, tag="ps", bufs=8, name=f"pg{t4}_{nt}")
                pb = psum.tile([128, 512], F32, tag="ps", bufs=8, name=f"pb{t4}_{nt}")
                for r in range(8):
                    nc.tensor.matmul(pg[:], wgb[:, r, ts(nt, 128)], et[:, r, :],
                                     start=(r == 0), stop=(r == 7))
                for r in range(8):
                    nc.tensor.matmul(pb[:], wgb[:, r, ts(4 + nt, 128)], et[:, r, :],
                                     start=(r == 0), stop=(r == 7))
                gate = sb2.tile([128, 512], BF16, tag="gate")
                nc.scalar.activation(gate[:], pg[:], AF.Sigmoid, bias=bgp[:, nt:nt + 1])
                uc = sb3.tile([128, 512], BF16, tag="uc")
                nc.vector.tensor_mul(uc[:], gate[:], pb[:])
                init = 0.0 if t4 == 0 else prev_states[:, nt, 511:512]
                nc.vector.tensor_tensor_scan(st[:, nt, :],
                                             apk[:, nt:nt + 1].to_broadcast([128, 512]),
                                             uc[:], init, OP.mult, OP.add)
            prev_states = st
        states = prev_states  # [128, 4, 512] bf16: my-window states

        # remaining persistent small tensors (first used in stage 3/4)
        wct = dma_in(sb, "wct", io["wct"].rearrange("(n p) h -> p n h", p=128), [128, 4, H], BF16)
        d1 = dma_in(sb, "d1", r8(io["d1"]), [128, 8, 3], F32)
        wocs = dma_in(sb, "wocs", r8(io["wocs"]), [128, 8, 3], F32)
        bfv = dma_in(sb, "bfv", r8(io["bfv"]), [128, 8, 1], F32)

        # ---- stage 3 with DEFERRED LayerNorm: Wo runs on raw y; the LN scaling
        # folds into the evacuation:  o = (y@Wo')*rstd + mur*(-colsum(Wo')),
        # since normed = y*rstd - mur and mur = mean*rstd.  The LN stats chain
        # (ACT/DVE) therefore never blocks the PE.  bop folds into bf on host.
        ys, sqs, lnstats = [None] * 3, [None] * 3, [None] * 3

        def emit_y(i):
            ys[i], sqs[i] = [], []
            tls = NT_OF_LAYER[i]
            for ho in range(8):
                py = psum.tile([128, TPC], F32, tag="ps", bufs=8, name=f"py{i}_{ho}")
                for j, nt in enumerate(tls):
                    nc.tensor.matmul(py[:], wct[:, nt, ts(ho, 128)], states[:, nt, :],
                                     start=(j == 0), stop=(j == len(tls) - 1))
                y = sb2.tile([128, TPC], BF16, tag="yt", bufs=16, name=f"y{i}_{ho}")
                nc.vector.scalar_tensor_tensor(y[:], embm[:, ho, :], d1[:, ho, i:i + 1],
                                               py[:], OP.mult, OP.add)
                s = sb2.tile([128, TPC], BF16, tag="sqt", bufs=16, name=f"s{i}_{ho}")
                nc.scalar.activation(s[:], y[:], AF.Square)
                ys[i].append(y)
                sqs[i].append(s)

        cmb = [None] * 24  # 24 per-k [128, TPC] bf16 tiles (fine-grained deps into Wf)

        def emit_wo(i):
            rstd, mur = lnstats[i]
            for ho in range(8):
                woth = sb2.tile([128, 24, 128], BF16, tag="wst", name=f"woth{i}_{ho}")[:, :8, :]
                nc.sync.dma_start(woth[:], r8(io["wot"])[:, :, ds(i * H + ho * 128, 128)])
                po = psum.tile([128, TPC], F32, tag="ps", bufs=8, name=f"po{i}_{ho}")
                for hi in range(8):
                    nc.tensor.matmul(po[:], woth[:, hi, :], ys[i][hi][:],
                                     start=(hi == 0), stop=(hi == 7))
                x1 = sb2.tile([128, TPC], F32, tag="tmp", bufs=3, name=f"tp{i}_{ho}")
                nc.vector.tensor_mul(x1[:], po[:], rstd[:])
                c = sb.tile([128, TPC], BF16, tag="cmb", bufs=24, name=f"cmb{i}_{ho}")
                nc.vector.scalar_tensor_tensor(c[:], mur[:], wocs[:, ho, i:i + 1], x1[:],
                                               OP.mult, OP.add)
                cmb[i * 8 + ho] = c

        emit_y(0)
        emit_y(1)
        lnstats[0] = _layernorm_stats(nc, psum, sb, ones, epst, ys[0], sqs[0], "l0")
        emit_wo(0)
        emit_y(2)
        lnstats[1] = _layernorm_stats(nc, psum, sb, ones, epst, ys[1], sqs[1], "l1")
        emit_wo(1)
        lnstats[2] = _layernorm_stats(nc, psum, sb, ones, epst, ys[2], sqs[2], "l2")
        emit_wo(2)

        # ---- stage 4: fused = LN2(combined@WfT + bf' + emb); mean subtracted
        # before the head matmul, rstd2 applied (transposed) at head evacuation.
        fs, sq2s = [], []
        for ho in range(8):
            wfh = sb2.tile([128, 24, 128], BF16, tag="wst", name=f"wfh{ho}")
            nc.sync.dma_start(wfh[:], io["wft"].rearrange("(k p) m -> p k m", p=128)[:, :, ts(ho, 128)])
            pf = psum.tile([128, TPC], F32, tag="ps", bufs=8, name=f"pf{ho}")
            for k in range(24):
                nc.tensor.matmul(pf[:], wfh[:, k, :], cmb[k][:],
                                 start=(k == 0), stop=(k == 23))
            f = sb2.tile([128, TPC], BF16, tag="yt", bufs=16, name=f"f{ho}")
            nc.vector.scalar_tensor_tensor(f[:], pf[:], bfv[:, ho, 0:1],
                                           embm[:, ho, :], OP.add, OP.add)
            s2 = sb2.tile([128, TPC], BF16, tag="sqt", bufs=16, name=f"s4_{ho}")
            nc.scalar.activation(s2[:], f[:], AF.Square)
            fs.append(f)
            sq2s.append(s2)
        pm1f = psum.tile([128, TPC], F32, tag="ps", bufs=8, name="pm1f")
        pm2f = psum.tile([128, TPC], F32, tag="ps", bufs=8, name="pm2f")
        for ho in range(8):
            nc.tensor.matmul(pm1f[:], ones[:], fs[ho][:], start=(ho == 0), stop=(ho == 7))
        for ho in range(8):
            nc.tensor.matmul(pm2f[:], ones[:], sq2s[ho][:], start=(ho == 0), stop=(ho == 7))
        mub = sb.tile([128, TPC], F32, tag="mub")
        nc.scalar.activation(mub[:], pm1f[:], AF.Copy, scale=1.0 / H)
        fc = []
        for ho in range(8):
            nv = sb.tile([128, TPC], BF16, tag="n2", bufs=8, name=f"fc{ho}")
            nc.vector.tensor_sub(nv[:], fs[ho][:], mub[:])
            fc.append(nv)
        # rstd2 = 1/sqrt(E[f^2] - mu^2 + eps), then transpose to token-major
        musqf = sb.tile([128, TPC], F32, tag="musq", bufs=2, name="musqf")
        nc.scalar.activation(musqf[:], pm1f[:], AF.Square, scale=1.0 / H)
        varf = sb.tile([128, TPC], F32, tag="var", bufs=2, name="varf")
        nc.vector.scalar_tensor_tensor(varf[:], pm2f[:], 1.0 / H, musqf[:], OP.mult, OP.subtract)
        sdf = sb.tile([128, TPC], F32, tag="sd", bufs=2, name="sdf")
        nc.scalar.activation(sdf[:], varf[:], AF.Sqrt, bias=epst[:, 0:1])
        rstd2 = sb.tile([128, TPC], F32, tag="rstd", bufs=2, name="rstd2")
        nc.vector.reciprocal(rstd2[:], sdf[:])
        ident = sb.tile([128, 128], F32, tag="ident")
        make_identity(nc, ident[:])
        rstd2T = sb.tile([128, 4], F32, tag="rstd2T")
        for t in range(4):
            ptp = psum.tile([128, 128], F32, tag="ps", bufs=8, name=f"ptp{t}")
            nc.tensor.transpose(ptp[:], rstd2[:, ts(t, 128)], ident[:])
            nc.scalar.copy(rstd2T[:, t:t + 1], ptp[:, 0:1])

        # ---- stage 5: head: logits[t, v] = (fc.T @ Wh') * rstd2T (+ bh') ----
        # bh' rides an extra accumulation row of sd2 values (pre-scale cancels
        # the rstd2 applied at evacuation).
        bh_nonzero = io.get("bhp") is not None
        if bh_nonzero:
            e0sd = sb.tile([128, 4, 128], BF16, tag="e0sd")
            nc.gpsimd.memset(e0sd[:], 0.0)
            for t in range(4):
                nc.vector.tensor_copy(e0sd[0:1, t, :], sdf[0:1, ts(t, 128)])
        for grp in range(NGRP):
            whg = sb2.tile([128, 8, VG], BF16, tag="w16", bufs=3, name=f"whg{grp}")
            nc.sync.dma_start(whg[:], r8(io["wht"])[:, :, ts(grp, VG)])
            if bh_nonzero:
                bhg = sb2.tile([128, VG], BF16, tag="bhg")
                nc.gpsimd.memset(bhg[:], 0.0)
                nc.sync.dma_start(bhg[0:1, :], io["bhp"][:, ts(grp, VG)])
            for t in range(4):
                osb = sb2.tile([128, VG], F32, tag="osb")
                pcs = [psum.tile([128, VC], F32, tag="ps", bufs=8, name=f"hc{grp}_{t}_{c}")
                       for c in range(NCHK)]
                for k in range(8):
                    for c in range(NCHK):
                        nc.tensor.matmul(pcs[c][:], fc[k][:, ts(t, 128)], whg[:, k, ts(c, VC)],
                                         start=(k == 0), stop=(k == 7 and not bh_nonzero))
                if bh_nonzero:
                    for c in range(NCHK):
                        nc.tensor.matmul(pcs[c][:], e0sd[:, t, :], bhg[:, ts(c, VC)],
                                         start=False, stop=True)
                for c in range(NCHK):
                    if c % 2 == 0:
                        nc.scalar.mul(osb[:, ts(c, VC)], pcs[c][:], rstd2T[:, t:t + 1])
                    else:
                        nc.vector.tensor_scalar_mul(osb[:, ts(c, VC)], pcs[c][:], rstd2T[:, t:t + 1])
                nc.sync.dma_start(io["out"][ts(t, 128), ts(grp, VG)], osb[:])


def _get_program(bh_nonzero):
    key = bool(bh_nonzero)
    if key in _prog_cache:
        return _prog_cache[key]
    nc = bacc.Bacc("TRN2", target_bir_lowering=False, debug=False, num_devices=8)
    io = {}

    def din(name, shape, dtype):
        io[name] = nc.dram_tensor(name, list(shape), dtype, kind="ExternalInput").ap()

    din("embt", (H, S), BF16)
    din("embm", (H, TPC), BF16)
    din("wgb", (H, 2 * NP), BF16)
    din("bgp", (NP, 1), F32)
    din("apk", (NP, 1), F32)
    din("wct", (NP, H), BF16)
    din("d1", (H, 3), F32)
    din("wot", (H, 3 * H), BF16)
    din("wocs", (H, 3), F32)
    din("wft", (3 * H, H), BF16)
    din("bfv", (H, 1), F32)
    din("wht", (H, V), BF16)
    if bh_nonzero:
        din("bhp", (1, V), BF16)
    else:
        io["bhp"] = None
    io["out"] = nc.dram_tensor("out", [TPC, V], F32, kind="ExternalOutput").ap()

    with tile.TileContext(nc) as tc:
        _body(tc, io)
    nc.compile()
    _prog_cache[key] = nc
    return nc


def _prepare(inputs):
    f32 = np.float32
    E = np.asarray(inputs["E"], f32)
    x = np.asarray(inputs["x"]).astype(np.int64)
    emb = E[x]  # [B, S, H]

    Wg = [np.asarray(inputs[f"Wg{i}"], f32) for i in range(3)]
    Wb = [np.asarray(inputs[f"Wb{i}"], f32) for i in range(3)]
    Wc = [np.asarray(inputs[f"Wc{i}"], f32) for i in range(3)]
    Dv = [np.asarray(inputs[f"D{i}"], f32) for i in range(3)]
    bg = [np.asarray(inputs[f"bg{i}"], f32) for i in range(3)]
    Wo = [np.asarray(inputs[f"Wo{i}"], f32) for i in range(3)]
    bo = [np.asarray(inputs[f"bo{i}"], f32) for i in range(3)]
    gv = [np.asarray(inputs[f"g{i}"], f32) for i in range(3)]
    be = [np.asarray(inputs[f"be{i}"], f32) for i in range(3)]
    Alog = [np.asarray(inputs[f"A{i}"], f32) for i in range(3)]
    Wf = np.asarray(inputs["Wf"], f32)
    bf = np.asarray(inputs["bf"], f32)
    gf = np.asarray(inputs["gf"], f32)
    bef = np.asarray(inputs["bef"], f32)
    Wh = np.asarray(inputs["Wh"], f32)
    bh = np.asarray(inputs["bh"], f32)

    wgb = np.zeros((H, 2 * NP), f32)
    bgp = np.zeros((NP, 1), f32)
    apk = np.zeros((NP, 1), f32)
    wct = np.zeros((NP, H), f32)
    for i, N in enumerate(SDS):
        s = SLOT[i]
        wgb[:, s:s + N] = Wg[i].T
        wgb[:, NP + s:NP + s + N] = Wb[i].T
        bgp[s:s + N, 0] = bg[i]
        apk[s:s + N, 0] = np.exp(Alog[i])
        wct[s:s + N, :] = Wc[i].T
    d1 = np.stack([Dv[i] + 1.0 for i in range(3)], axis=1)  # [H, 3]
    wop = [(Wo[i] * gv[i][None, :]) for i in range(3)]
    wot = np.concatenate([w.T for w in wop], axis=1)  # [H, 3H]
    # deferred-LN evac term: o = (y@Wo')*rstd + mur*(-colsum(Wo'))
    wocs = np.stack([-w.sum(axis=1) for w in wop], axis=1)  # [H, 3]
    # per-layer output bias (Wo@be + bo) folds into the Wf bias
    bo_cat = np.concatenate([Wo[i] @ be[i] + bo[i] for i in range(3)])  # [3H]
    bfp = bf + Wf @ bo_cat  # [H]
    wft = Wf.T.copy()  # [3H, H]
    wht = (Wh * gf[None, :]).T.copy()  # [H, V]
    bhp = Wh @ bef + bh  # [V]
    bh_nonzero = bool(np.abs(bhp).max() > 0)

    shared = {
        "wgb": wgb.astype(BF),
        "bgp": bgp,
        "apk": apk,
        "wct": wct.astype(BF),
        "d1": d1,
        "wot": wot.astype(BF),
        "wocs": wocs,
        "wft": wft.astype(BF),
        "bfv": bfp.reshape(H, 1),
        "wht": wht.astype(BF),
    }
    if bh_nonzero:
        shared["bhp"] = bhp.reshape(1, V).astype(BF)

    in_maps = []
    for c in range(8):
        b, q = c // 4, c % 4
        plen = TPC * (q + 1)
        et = np.zeros((H, S), BF)
        et[:, S - plen:] = emb[b, :plen].T.astype(BF)
        em = np.ascontiguousarray(emb[b, q * TPC:(q + 1) * TPC].T).astype(BF)
        m = dict(shared)
        m["embt"] = et
        m["embm"] = em
        in_maps.append(m)
    return in_maps, bh_nonzero


def kernel(**inputs):
    global last_exec_time_ns, last_bass_results
    in_maps, bh_nonzero = _prepare(inputs)
    nc = _get_program(bh_nonzero)
    trace = os.environ.get("BASS_KERNEL_TRACE") == "1"
    tmpdir = os.environ.get("BASS_TRACE_DIR") or None
    res = run_bass_kernel_spmd(nc, in_maps, core_ids=list(range(8)), trace=trace,
                               tmpdir=tmpdir)
    last_exec_time_ns = res.exec_time_ns
    last_bass_results = res

    out = np.empty((B, S, V), np.float32)
    for c in range(8):
        b, q = c // 4, c % 4
        out[b, q * TPC:(q + 1) * TPC, :] = res.results[c]["out"]
    return out


# revision 36
# speedup vs baseline: 1.1094x; 1.0565x over previous
"""Trainium2 Bass kernel for nn_MultiScaleDHSM (multi-scale diagonal-SSM LM block).

Strategy (zero-communication SPMD over 8 cores):
  core c owns tokens [512*q, 512*(q+1)) of batch b, where b=c//4, q=c%4.
  Device keeps everything feature-major [feature, token] so every per-feature
  vector (biases, D, LN folds) is a per-partition scalar.  The sequential
  recurrence s_t = A*s_{t-1} + u_t runs on the HW tensor_tensor_scan op over
  the core's right-aligned token prefix (front zero-padded so the 512-token
  "own window" sits at a static offset in an identical program on all cores).
  LayerNorm stats (reduction over features = partitions) are computed with a
  ones-matmul on the PE, which both reduces and broadcasts across partitions.
  LN scales fold into the following matmul weights on the host (g -> Wo,
  gf -> Wh); per-feature biases are applied as per-partition ACT biases.
"""

import os
from contextlib import ExitStack

import ml_dtypes
import numpy as np

import concourse.bass as bass
import concourse.mybir as mybir
import concourse.tile as tile
from concourse import bacc
from concourse.bass import ds, ts
from concourse.bass_utils import run_bass_kernel_spmd
from concourse.masks import make_identity

B, S, H, V = 2, 2048, 1024, 32000
SDS = [64, 128, 256]
NP = 512  # packed state dim: [L0:0-64 | pad:64-128 | L1:128-256 | L2:256-512]
SLOT = [0, 128, 256]
NT_OF_LAYER = [[0], [1], [2, 3]]  # which 128-row n-tiles belong to each layer
TPC = 512  # tokens per core
EPS = 1e-5
VG = 1000  # head vocab group (streamed Wh slice width)
VC = 500   # head vocab chunk (one PSUM bank, <=512 fp32)
NGRP = V // VG
NCHK = VG // VC

F32 = mybir.dt.float32
BF16 = mybir.dt.bfloat16
BF = ml_dtypes.bfloat16
AF = mybir.ActivationFunctionType
OP = mybir.AluOpType

last_exec_time_ns = None
last_bass_results = None

_prog_cache = {}


def _layernorm_stats(nc, psum, sb, ones, epst, ys, sqs, suf):
    """ys/sqs: lists of 8 [128, TPC] bf16 tiles. Returns (rstd, mur) [128, TPC] f32."""
    pm1 = psum.tile([128, TPC], F32, tag="ps", bufs=8, name=f"pm1{suf}")
    pm2 = psum.tile([128, TPC], F32, tag="ps", bufs=8, name=f"pm2{suf}")
    for ho in range(8):
        nc.tensor.matmul(pm1[:], ones[:], ys[ho][:], start=(ho == 0), stop=(ho == 7))
    for ho in range(8):
        nc.tensor.matmul(pm2[:], ones[:], sqs[ho][:], start=(ho == 0), stop=(ho == 7))
    musq = sb.tile([128, TPC], F32, tag="musq", bufs=2, name=f"musq{suf}")
    nc.scalar.activation(musq[:], pm1[:], AF.Square, scale=1.0 / H)
    var = sb.tile([128, TPC], F32, tag="var", bufs=2, name=f"var{suf}")
    nc.vector.scalar_tensor_tensor(var[:], pm2[:], 1.0 / H, musq[:], OP.mult, OP.subtract)
    sd = sb.tile([128, TPC], F32, tag="sd", bufs=2, name=f"sd{suf}")
    nc.scalar.activation(sd[:], var[:], AF.Sqrt, bias=epst[:, 0:1])
    rstd = sb.tile([128, TPC], F32, tag="rstd", bufs=2, name=f"rstd{suf}")
    nc.vector.reciprocal(rstd[:], sd[:])
    mur = sb.tile([128, TPC], F32, tag="mur", bufs=2, name=f"mur{suf}")
    nc.vector.scalar_tensor_tensor(mur[:], pm1[:], 1.0 / H, rstd[:], OP.mult, OP.mult)
    return rstd, mur


def _body(tc, io):
    nc = tc.nc
    with ExitStack() as ctx:
        sb = ctx.enter_context(tc.tile_pool(name="sb", bufs=1))
        sb2 = ctx.enter_context(tc.tile_pool(name="sb2", bufs=2))
        sb3 = ctx.enter_context(tc.tile_pool(name="sb3", bufs=3))
        psum = ctx.enter_context(tc.tile_pool(name="ps", bufs=4, space="PSUM"))

        r8 = lambda ap: ap.rearrange("(r p) t -> p r t", p=128)

        def dma_in(pool, name, src_ap, shape, dtype, bufs=None):
            kw = {"bufs": bufs} if bufs else {}
            t = pool.tile(shape, dtype, tag=name, name=name, **kw)
            nc.sync.dma_start(t[:], src_ap)
            return t

        # ---- persistent small tensors (stage-1 needs first; rest after) ----
        wgb = sb2.tile([128, 8, 2 * NP], BF16, tag="w16", bufs=3, name="wgb")
        for r in range(8):
            nc.sync.dma_start(wgb[:, r, :], r8(io["wgb"])[:, r, :])
        bgp = dma_in(sb, "bgp", io["bgp"].rearrange("(n p) o -> p (n o)", p=128), [128, 4], F32)
        apk = dma_in(sb, "apk", io["apk"].rearrange("(n p) o -> p (n o)", p=128), [128, 4], F32)
        ones = sb.tile([128, 128], BF16, tag="ones")
        nc.gpsimd.memset(ones[:], 1.0)
        epst = sb.tile([128, 1], F32, tag="epst")
        nc.gpsimd.memset(epst[:], EPS)

        # ---- stage 1+2: u = sigmoid(emb@WgT + bg) * (emb@WbT); chained HW scan ----
        states = None
        prev_states = None
        embm = None  # the t4=3 chunk doubles as the own-window embedding
        for t4 in range(4):
            if t4 < 3:
                et = sb2.tile([128, 8, 512], BF16, tag="e8", name=f"et{t4}")
            else:
                et = sb.tile([128, 8, 512], BF16, tag="embm", name="et3")
                embm = et
            for r in range(8):
                nc.sync.dma_start(et[:, r, :], r8(io["embt"])[:, r, ts(t4, 512)])
            st = sb2.tile([128, 4, 512], BF16, tag="stc", name=f"st{t4}")
            for nt in range(4):
                pg = psum.tile([128, 512], F32, tag="ps", bufs=8, name=f"pg{t4}_{nt}")
                pb = psum.tile([128, 512], F32, tag="ps", bufs=8, name=f"pb{t4}_{nt}")
                for r in range(8):
                    nc.tensor.matmul(pg[:], wgb[:, r, ts(nt, 128)], et[:, r, :],
                                     start=(r == 0), stop=(r == 7))
                for r in range(8):
                    nc.tensor.matmul(pb[:], wgb[:, r, ts(4 + nt, 128)], et[:, r, :],
                                     start=(r == 0), stop=(r == 7))
                gate = sb2.tile([128, 512], BF16, tag="gate", bufs=1)
                nc.scalar.activation(gate[:], pg[:], AF.Sigmoid, bias=bgp[:, nt:nt + 1])
                uc = sb3.tile([128, 512], BF16, tag="uc")
                nc.vector.tensor_mul(uc[:], gate[:], pb[:])
                init = 0.0 if t4 == 0 else prev_states[:, nt, 511:512]
                nc.vector.tensor_tensor_scan(st[:, nt, :],
                                             apk[:, nt:nt + 1].to_broadcast([128, 512]),
                                             uc[:], init, OP.mult, OP.add)
            prev_states = st
        states = prev_states  # [128, 4, 512] bf16: my-window states

        # remaining persistent small tensors (first used in stage 3/4)
        wct = dma_in(sb, "wct", io["wct"].rearrange("(n p) h -> p n h", p=128), [128, 4, H], BF16)
        d1 = dma_in(sb, "d1", r8(io["d1"]), [128, 8, 3], F32)
        wocs = dma_in(sb, "wocs", r8(io["wocs"]), [128, 8, 3], F32)
        bfv = dma_in(sb, "bfv", r8(io["bfv"]), [128, 8, 1], F32)

        # ---- stage 3 with DEFERRED LayerNorm: Wo runs on raw y; the LN scaling
        # folds into the evacuation:  o = (y@Wo')*rstd + mur*(-colsum(Wo')),
        # since normed = y*rstd - mur and mur = mean*rstd.  The LN stats chain
        # (ACT/DVE) therefore never blocks the PE.  bop folds into bf on host.
        ys, sqs, lnstats = [None] * 3, [None] * 3, [None] * 3

        def emit_y(i):
            ys[i], sqs[i] = [], []
            tls = NT_OF_LAYER[i]
            for ho in range(8):
                py = psum.tile([128, TPC], F32, tag="ps", bufs=8, name=f"py{i}_{ho}")
                for j, nt in enumerate(tls):
                    nc.tensor.matmul(py[:], wct[:, nt, ts(ho, 128)], states[:, nt, :],
                                     start=(j == 0), stop=(j == len(tls) - 1))
                y = sb2.tile([128, TPC], BF16, tag="yt", bufs=16, name=f"y{i}_{ho}")
                nc.vector.scalar_tensor_tensor(y[:], embm[:, ho, :], d1[:, ho, i:i + 1],
                                               py[:], OP.mult, OP.add)
                s = sb2.tile([128, TPC], BF16, tag="sqt", bufs=16, name=f"s{i}_{ho}")
                nc.scalar.activation(s[:], y[:], AF.Square)
                ys[i].append(y)
                sqs[i].append(s)

        cmb = [None] * 24  # 24 per-k [128, TPC] bf16 tiles (fine-grained deps into Wf)

        def emit_wo(i):
            rstd, mur = lnstats[i]
            for ho in range(8):
                woth = sb2.tile([128, 24, 128], BF16, tag="wst", name=f"woth{i}_{ho}")[:, :8, :]
                nc.scalar.dma_start(woth[:], r8(io["wot"])[:, :, ds(i * H + ho * 128, 128)])
                po = psum.tile([128, TPC], F32, tag="ps", bufs=8, name=f"po{i}_{ho}")
                for hi in range(8):
                    nc.tensor.matmul(po[:], woth[:, hi, :], ys[i][hi][:],
                                     start=(hi == 0), stop=(hi == 7))
                x1 = sb2.tile([128, TPC], F32, tag="tmp", bufs=2, name=f"tp{i}_{ho}")
                nc.vector.tensor_mul(x1[:], po[:], rstd[:])
                c = sb.tile([128, TPC], BF16, tag="cmb", bufs=24, name=f"cmb{i}_{ho}")
                nc.vector.scalar_tensor_tensor(c[:], mur[:], wocs[:, ho, i:i + 1], x1[:],
                                               OP.mult, OP.add)
                cmb[i * 8 + ho] = c

        emit_y(0)
        emit_y(1)
        lnstats[0] = _layernorm_stats(nc, psum, sb, ones, epst, ys[0], sqs[0], "l0")
        emit_wo(0)
        emit_y(2)
        lnstats[1] = _layernorm_stats(nc, psum, sb, ones, epst, ys[1], sqs[1], "l1")
        emit_wo(1)
        lnstats[2] = _layernorm_stats(nc, psum, sb, ones, epst, ys[2], sqs[2], "l2")
        emit_wo(2)

        # ---- stage 4: fused = LN2(combined@WfT + bf' + emb); mean subtracted
        # before the head matmul, rstd2 applied (transposed) at head evacuation.
        fs, sq2s = [], []
        for ho in range(8):
            wfh = sb2.tile([128, 24, 128], BF16, tag="wst", name=f"wfh{ho}")
            nc.scalar.dma_start(wfh[:], io["wft"].rearrange("(k p) m -> p k m", p=128)[:, :, ts(ho, 128)])
            pf = psum.tile([128, TPC], F32, tag="ps", bufs=8, name=f"pf{ho}")
            for k in range(24):
                nc.tensor.matmul(pf[:], wfh[:, k, :], cmb[k][:],
                                 start=(k == 0), stop=(k == 23))
            f = sb2.tile([128, TPC], BF16, tag="yt", bufs=16, name=f"f{ho}")
            nc.vector.scalar_tensor_tensor(f[:], pf[:], bfv[:, ho, 0:1],
                                           embm[:, ho, :], OP.add, OP.add)
            s2 = sb2.tile([128, TPC], BF16, tag="sqt", bufs=16, name=f"s4_{ho}")
            nc.scalar.activation(s2[:], f[:], AF.Square)
            fs.append(f)
            sq2s.append(s2)
        pm1f = psum.tile([128, TPC], F32, tag="ps", bufs=8, name="pm1f")
        pm2f = psum.tile([128, TPC], F32, tag="ps", bufs=8, name="pm2f")
        for ho in range(8):
            nc.tensor.matmul(pm1f[:], ones[:], fs[ho][:], start=(ho == 0), stop=(ho == 7))
        for ho in range(8):
            nc.tensor.matmul(pm2f[:], ones[:], sq2s[ho][:], start=(ho == 0), stop=(ho == 7))
        mub = sb.tile([128, TPC], F32, tag="mub")
        nc.scalar.activation(mub[:], pm1f[:], AF.Copy, scale=1.0 / H)
        fc = []
        for ho in range(8):
            nv = sb.tile([128, TPC], BF16, tag="n2", bufs=8, name=f"fc{ho}")
            nc.vector.tensor_sub(nv[:], fs[ho][:], mub[:])
            fc.append(nv)
        # rstd2 = 1/sqrt(E[f^2] - mu^2 + eps), then transpose to token-major
        musqf = sb.tile([128, TPC], F32, tag="musq", bufs=2, name="musqf")
        nc.scalar.activation(musqf[:], pm1f[:], AF.Square, scale=1.0 / H)
        varf = sb.tile([128, TPC], F32, tag="var", bufs=2, name="varf")
        nc.vector.scalar_tensor_tensor(varf[:], pm2f[:], 1.0 / H, musqf[:], OP.mult, OP.subtract)
        sdf = sb.tile([128, TPC], F32, tag="sd", bufs=2, name="sdf")
        nc.scalar.activation(sdf[:], varf[:], AF.Sqrt, bias=epst[:, 0:1])
        rstd2 = sb.tile([128, TPC], F32, tag="rstd", bufs=2, name="rstd2")
        nc.vector.reciprocal(rstd2[:], sdf[:])
        ident = sb.tile([128, 128], F32, tag="ident")
        make_identity(nc, ident[:])
        rstd2T = sb.tile([128, 4], F32, tag="rstd2T")
        for t in range(4):
            ptp = psum.tile([128, 128], F32, tag="ps", bufs=8, name=f"ptp{t}")
            nc.tensor.transpose(ptp[:], rstd2[:, ts(t, 128)], ident[:])
            nc.scalar.copy(rstd2T[:, t:t + 1], ptp[:, 0:1])

        # ---- stage 5: head: logits[t, v] = (fc.T @ Wh') * rstd2T (+ bh') ----
        # bh' rides an extra accumulation row of sd2 values (pre-scale cancels
        # the rstd2 applied at evacuation).
        bh_nonzero = io.get("bhp") is not None
        if bh_nonzero:
            e0sd = sb.tile([128, 4, 128], BF16, tag="e0sd")
            nc.gpsimd.memset(e0sd[:], 0.0)
            for t in range(4):
                nc.vector.tensor_copy(e0sd[0:1, t, :], sdf[0:1, ts(t, 128)])
        for grp in range(NGRP):
            whg = sb2.tile([128, 8, VG], BF16, tag="w16", bufs=3, name=f"whg{grp}")
            nc.sync.dma_start(whg[:], r8(io["wht"])[:, :, ts(grp, VG)])
            if bh_nonzero:
                bhg = sb2.tile([128, VG], BF16, tag="bhg")
                nc.gpsimd.memset(bhg[:], 0.0)
                nc.sync.dma_start(bhg[0:1, :], io["bhp"][:, ts(grp, VG)])
            for t in range(4):
                osb = sb2.tile([128, VG], F32, tag="osb", bufs=3)
                pcs = [psum.tile([128, VC], F32, tag="ps", bufs=8, name=f"hc{grp}_{t}_{c}")
                       for c in range(NCHK)]
                for k in range(8):
                    for c in range(NCHK):
                        nc.tensor.matmul(pcs[c][:], fc[k][:, ts(t, 128)], whg[:, k, ts(c, VC)],
                                         start=(k == 0), stop=(k == 7 and not bh_nonzero))
                if bh_nonzero:
                    for c in range(NCHK):
                        nc.tensor.matmul(pcs[c][:], e0sd[:, t, :], bhg[:, ts(c, VC)],
                                         start=False, stop=True)
                for c in range(NCHK):
                    if c % 2 == 0:
                        nc.scalar.mul(osb[:, ts(c, VC)], pcs[c][:], rstd2T[:, t:t + 1])
                    else:
                        nc.vector.tensor_scalar_mul(osb[:, ts(c, VC)], pcs[c][:], rstd2T[:, t:t + 1])
                nc.gpsimd.dma_start(io["out"][ts(t, 128), ts(grp, VG)], osb[:])


def _get_program(bh_nonzero):
    key = bool(bh_nonzero)
    if key in _prog_cache:
        return _prog_cache[key]
    nc = bacc.Bacc("TRN2", target_bir_lowering=False, debug=False, num_devices=8)
    io = {}

    def din(name, shape, dtype):
        io[name] = nc.dram_tensor(name, list(shape), dtype, kind="ExternalInput").ap()

    din("embt", (H, S), BF16)
    din("embm", (H, TPC), BF16)
    din("wgb", (H, 2 * NP), BF16)
    din("bgp", (NP, 1), F32)
    din("apk", (NP, 1), F32)
    din("wct", (NP, H), BF16)
    din("d1", (H, 3), F32)
    din("wot", (H, 3 * H), BF16)
    din("wocs", (H, 3), F32)
    din("wft", (3 * H, H), BF16)
    din("bfv", (H, 1), F32)
    din("wht", (H, V), BF16)
    if bh_nonzero:
        din("bhp", (1, V), BF16)
    else:
        io["bhp"] = None
    io["out"] = nc.dram_tensor("out", [TPC, V], F32, kind="ExternalOutput").ap()

    with tile.TileContext(nc) as tc:
        _body(tc, io)
    nc.compile()
    _prog_cache[key] = nc
    return nc


def _prepare(inputs):
    f32 = np.float32
    E = np.asarray(inputs["E"], f32)
    x = np.asarray(inputs["x"]).astype(np.int64)
    emb = E[x]  # [B, S, H]

    Wg = [np.asarray(inputs[f"Wg{i}"], f32) for i in range(3)]
    Wb = [np.asarray(inputs[f"Wb{i}"], f32) for i in range(3)]
    Wc = [np.asarray(inputs[f"Wc{i}"], f32) for i in range(3)]
    Dv = [np.asarray(inputs[f"D{i}"], f32) for i in range(3)]
    bg = [np.asarray(inputs[f"bg{i}"], f32) for i in range(3)]
    Wo = [np.asarray(inputs[f"Wo{i}"], f32) for i in range(3)]
    bo = [np.asarray(inputs[f"bo{i}"], f32) for i in range(3)]
    gv = [np.asarray(inputs[f"g{i}"], f32) for i in range(3)]
    be = [np.asarray(inputs[f"be{i}"], f32) for i in range(3)]
    Alog = [np.asarray(inputs[f"A{i}"], f32) for i in range(3)]
    Wf = np.asarray(inputs["Wf"], f32)
    bf = np.asarray(inputs["bf"], f32)
    gf = np.asarray(inputs["gf"], f32)
    bef = np.asarray(inputs["bef"], f32)
    Wh = np.asarray(inputs["Wh"], f32)
    bh = np.asarray(inputs["bh"], f32)

    wgb = np.zeros((H, 2 * NP), f32)
    bgp = np.zeros((NP, 1), f32)
    apk = np.zeros((NP, 1), f32)
    wct = np.zeros((NP, H), f32)
    for i, N in enumerate(SDS):
        s = SLOT[i]
        wgb[:, s:s + N] = Wg[i].T
        wgb[:, NP + s:NP + s + N] = Wb[i].T
        bgp[s:s + N, 0] = bg[i]
        apk[s:s + N, 0] = np.exp(Alog[i])
        wct[s:s + N, :] = Wc[i].T
    d1 = np.stack([Dv[i] + 1.0 for i in range(3)], axis=1)  # [H, 3]
    wop = [(Wo[i] * gv[i][None, :]) for i in range(3)]
    wot = np.concatenate([w.T for w in wop], axis=1)  # [H, 3H]
    # deferred-LN evac term: o = (y@Wo')*rstd + mur*(-colsum(Wo'))
    wocs = np.stack([-w.sum(axis=1) for w in wop], axis=1)  # [H, 3]
    # per-layer output bias (Wo@be + bo) folds into the Wf bias
    bo_cat = np.concatenate([Wo[i] @ be[i] + bo[i] for i in range(3)])  # [3H]
    bfp = bf + Wf @ bo_cat  # [H]
    wft = Wf.T.copy()  # [3H, H]
    wht = (Wh * gf[None, :]).T.copy()  # [H, V]
    bhp = Wh @ bef + bh  # [V]
    bh_nonzero = bool(np.abs(bhp).max() > 0)

    shared = {
        "wgb": wgb.astype(BF),
        "bgp": bgp,
        "apk": apk,
        "wct": wct.astype(BF),
        "d1": d1,
        "wot": wot.astype(BF),
        "wocs": wocs,
        "wft": wft.astype(BF),
        "bfv": bfp.reshape(H, 1),
        "wht": wht.astype(BF),
    }
    if bh_nonzero:
        shared["bhp"] = bhp.reshape(1, V).astype(BF)

    in_maps = []
    for c in range(8):
        b, q = c // 4, c % 4
        plen = TPC * (q + 1)
        et = np.zeros((H, S), BF)
        et[:, S - plen:] = emb[b, :plen].T.astype(BF)
        em = np.ascontiguousarray(emb[b, q * TPC:(q + 1) * TPC].T).astype(BF)
        m = dict(shared)
        m["embt"] = et
        m["embm"] = em
        in_maps.append(m)
    return in_maps, bh_nonzero


def kernel(**inputs):
    global last_exec_time_ns, last_bass_results
    in_maps, bh_nonzero = _prepare(inputs)
    nc = _get_program(bh_nonzero)
    trace = os.environ.get("BASS_KERNEL_TRACE") == "1"
    tmpdir = os.environ.get("BASS_TRACE_DIR") or None
    res = run_bass_kernel_spmd(nc, in_maps, core_ids=list(range(8)), trace=trace,
                               tmpdir=tmpdir)
    last_exec_time_ns = res.exec_time_ns
    last_bass_results = res

    out = np.empty((B, S, V), np.float32)
    for c in range(8):
        b, q = c // 4, c % 4
        out[b, q * TPC:(q + 1) * TPC, :] = res.results[c]["out"]
    return out
